# revision 44
# baseline (speedup 1.0000x reference)
"""Trainium2 Bass kernel for a dense transformer block (RMSNorm -> causal MHA
-> residual -> RMSNorm -> GLU FFN -> residual), SPMD across 8 NeuronCores.

Sharding: tensor-parallel attention (2 heads/core) -> AllToAll of per-head
attention outputs -> token-parallel proj + FFN (512 tokens/core).

v3 design (sim-driven; HW-verified constants from v2 retained).
HW-measured (loop-contrast k=2 vs 182): body 180.5us no-collective,
195.5us with the A2A estimate, vs the v2 baseline's 485us; rel-err
0.0145 vs the 2e-2 gate. TimelineSim (cold single pass) 276us; the
hardware loop pipelines across iterations so steady-state is lower.

 - the QKV streaming loop and causal attention are MERGED: chunk r's
   QKV/stats matmuls are emission-interleaved with chunk r-1's attention
   pairs, so the PE never idles waiting for the Act exp chain and stays
   at full p-state (2.4 GHz); attention alone is Act-exp-bound.
 - RMS1 stats run one chunk AHEAD of their QKV consumers (the serial
   Square->reduce->exp->scale chain gets a whole iteration of slack) and
   the DoubleRow ones-reduce uses M=128 (fp8 dual-row Ldweights with
   M=8 fails the s3_lw_dual_fp8_restrictions ISA check; M=128 also makes
   every ss row the full sum, so the partition broadcast is free).
 - the whole merged phase fits in 8 PSUM banks: per-head f32 score
   tiles (4) + one rotating QKV accumulator (1) + a shared
   ss/nbc/tp/bc scratch bank (1) + the two AV accumulators (2).
 - rs chain via exp(0.25*(v-2)^2-0.25) ~ rsqrt(v): Square+Exp share the
   attention exp act table -> no ACT_TABLE_LOADs inside the merged loop
   (Sqrt or Ln would force 2 reloads per chunk, ~24us total).
 - PSUM->SBUF copies (vv, xn2T8) on DVE, not Act (Act is the
   attention-critical engine; Pool/GpSimd has no PSUM access); diagonal
   mask muls on Pool (SBUF-only, keeps the in-order DVE queue free for
   the softmax-denominator tail so deferred AVs unblock sooner).
 - FFN: w1a as fp8 hi+lo DoubleRow against the existing fp8 xn2 (2
   passes, 2x rate), glu stored as fp8 hi+lo, w2 as 3-term hi/lo
   product. Offline numerics: rel-err 0.0144 vs 0.0135 baseline.
"""
import numpy as np
import ml_dtypes

import concourse.bass as bass
import concourse.tile as tile
from concourse import bacc, mybir
from concourse.bass_utils import run_bass_kernel_spmd
from concourse.masks import make_identity

F32 = mybir.dt.float32
BF16 = mybir.dt.bfloat16
FP8 = mybir.dt.float8e4
AF = mybir.ActivationFunctionType
ALU = mybir.AluOpType
PM = mybir.MatmulPerfMode

B, T, D, H, HD = 2, 2048, 1024, 16, 64
NCORES = 8
HPC = H // NCORES          # heads per core = 2
E2 = HPC * HD              # per-core attention channels = 128
NTOK = B * T               # 4096
TS = NTOK // NCORES        # tokens per core slice = 512
EPS = 1e-6
NDC = D // 128             # 8 D-chunks
NRC = NTOK // 512          # 8 token chunks
CORE_IDS = list(range(NCORES))
SX_SCALE = 16.0            # fp8 pre-scale for x.T hi/lo (QKV DoubleRow)
SW_SCALE = 256.0           # fp8 pre-scale for wq/wk/wv hi/lo
W1A_SCALE = 32.0           # fp8 pre-scale for w1 a-branch weights (hi+lo)
W1B_SCALE = 32.0           # fp8 pre-scale for w1 sigmoid-branch weights
W2_SCALE = 64.0            # fp8 pre-scale for w2 hi/lo split

_CACHE = {}


def _build(reps=1, variant="full", loop_k=0, stop_after=99):
    nc = bacc.Bacc("TRN2", target_bir_lowering=False, debug=False,
                   num_devices=NCORES)

    tensors = dict(
        xt8h=nc.dram_tensor("xt8h", [D, NTOK], FP8, kind="ExternalInput"),
        xt8l=nc.dram_tensor("xt8l", [D, NTOK], FP8, kind="ExternalInput"),
        wq8h=nc.dram_tensor("wq8h", [D, E2], FP8, kind="ExternalInput"),
        wq8l=nc.dram_tensor("wq8l", [D, E2], FP8, kind="ExternalInput"),
        wk8h=nc.dram_tensor("wk8h", [D, E2], FP8, kind="ExternalInput"),
        wk8l=nc.dram_tensor("wk8l", [D, E2], FP8, kind="ExternalInput"),
        wv8h=nc.dram_tensor("wv8h", [D, E2], FP8, kind="ExternalInput"),
        wv8l=nc.dram_tensor("wv8l", [D, E2], FP8, kind="ExternalInput"),
        wp=nc.dram_tensor("wp", [D, D], BF16, kind="ExternalInput"),
        w1ahi=nc.dram_tensor("w1ahi", [D, 4 * D], FP8, kind="ExternalInput"),
        w1alo=nc.dram_tensor("w1alo", [D, 4 * D], FP8, kind="ExternalInput"),
        w1b8=nc.dram_tensor("w1b8", [D, 4 * D], FP8, kind="ExternalInput"),
        b1=nc.dram_tensor("b1", [8 * D], F32, kind="ExternalInput"),
        w2hi=nc.dram_tensor("w2hi", [4 * D, D], FP8, kind="ExternalInput"),
        w2lo=nc.dram_tensor("w2lo", [4 * D, D], FP8, kind="ExternalInput"),
        b2bc=nc.dram_tensor("b2bc", [128, D], BF16, kind="ExternalInput"),
        xsp=nc.dram_tensor("xsp", [TS, D], F32, kind="ExternalInput"),
        mkt=nc.dram_tensor("mkt", [128, 128], BF16, kind="ExternalInput"),
        out=nc.dram_tensor("out", [TS, D], F32, kind="ExternalOutput"),
    )

    with tile.TileContext(nc) as tc:
        if loop_k:
            with tc.For_i(0, loop_k, 1):
                _body(nc, tc, tensors, variant=variant, stop_after=stop_after)
        else:
            for _ in range(reps):
                _body(nc, tc, tensors, variant=variant, stop_after=stop_after)
    nc.compile()
    return nc


def _finish(*pools):
    for p in pools:
        p.__exit__(None, None, None)


def _body(nc, tc, tn, variant="full", stop_after=99):
    ts = bass.ts
    xth_d, xtl_d, wp_d = tn["xt8h"], tn["xt8l"], tn["wp"]
    wqh_d, wql_d = tn["wq8h"], tn["wq8l"]
    wkh_d, wkl_d = tn["wk8h"], tn["wk8l"]
    wvh_d, wvl_d = tn["wv8h"], tn["wv8l"]
    w1ahi_d, w1alo_d, w1b8_d, b1_d = tn["w1ahi"], tn["w1alo"], tn["w1b8"], tn["b1"]
    w2hi_d, w2lo_d, b2bc_d = tn["w2hi"], tn["w2lo"], tn["b2bc"]
    xsp_d, mkt_d, out_d = tn["xsp"], tn["mkt"], tn["out"]

    persist = tc.tile_pool(name="persist", bufs=1)
    pp = persist.__enter__()
    dram = tc.tile_pool(name="dram", bufs=1, space="DRAM")
    dd = dram.__enter__()

    # ---- persistent small tensors ----
    eps_t = pp.tile([128, 1], F32)
    nc.vector.memset(eps_t, EPS)
    # warm the exp/square act table at t=0, overlapping the first DMAs
    # (otherwise the 1.3us ACT_TABLE_LOAD lands on the critical path)
    warm_t = pp.tile([128, 1], F32)
    nc.scalar.activation(out=warm_t, in_=eps_t, func=AF.Exp)
    # rs chain constants: recip8 = exp(0.25*(v-2)^2 - 0.25)/8 ~ rsqrt(v)/8
    # (3rd-order match at v=1; v=mean(x^2)+eps concentrates in [0.8,1.2] so
    # the error is ~t^3/6 < 2e-3, below the bf16 rounding of the result.
    # Uses only Square+Exp -> stays in the attention exp act table; Ln or
    # Sqrt here would force 2 ACT_TABLE_LOADs per chunk, ~24us total.)
    epsm2_t = pp.tile([128, 1], F32)
    nc.vector.memset(epsm2_t, EPS - 2.0)
    # nbc exp bias also folds 1/(SX_SCALE*SW_SCALE): the QKV accumulators
    # hold 4096x the true products (x shipped as fp8 hi/lo at x16, weights
    # at x256), and nbc is only ever used to scale those accumulators
    mq_t = pp.tile([128, 1], F32)
    nc.vector.memset(mq_t, -0.25 - float(np.log(SX_SCALE * SW_SCALE)))
    # stats DoubleRow lhsT (K=256, M=128): M=128 keeps the fp8 dual-row
    # Ldweights ISA-legal AND makes every row of ss the full column sum,
    # i.e. the partition broadcast comes for free
    ones128f8 = pp.tile([128, 2, 128], FP8)
    nc.vector.memset(ones128f8, 1.0)
    ones65 = pp.tile([65, HD], BF16)        # denom bcast lhsT at partition 64
    nc.vector.memset(ones65[64:65, :], 1.0)
    b1a_sb = pp.tile([128, 32], F32)
    b1b_sb = pp.tile([128, 32], F32)
    b2bc_sb = pp.tile([128, D], BF16)
    mkt_sb = pp.tile([128, 128], BF16)      # block-local triangle mask
    wp_sb = pp.tile([128, NDC, D], BF16)
    ident = pp.tile([128, 128], BF16)
    make_identity(nc, ident)
    # xsp_sb carries residual state through the whole kernel
    xsp_sb = pp.tile([128, 4, D], F32)

    a2a_in = dd.tile([NCORES, E2, TS], BF16)
    a2a_out = dd.tile([NCORES, E2, TS], BF16)

    # ---- FFN weight pool: opened FIRST (before the merged-phase pools)
    # so it can stay alive into the FFN while they close in LIFO order ----
    p5_cm = tc.tile_pool(name="p5", bufs=2)
    p5 = p5_cm.__enter__()

    # ---- tensors that span the merged phase ----
    span = tc.tile_pool(name="span", bufs=1)
    sp_ = span.__enter__()
    # NOTE: scores via fp8 DoubleRow (q/k packed as 64-partition k-tiles)
    # were tried and pass correctness (rel-err 0.0151) with -10us PE in
    # TimelineSim, but the looped HW build measured 1.53ms/body -- an
    # ~8x pathology the cost model does not capture (64-partition fp8
    # dual-row Ldweights or the SBUF->SBUF partition-remap DMAs).
    # Keeping the bf16 zero-padded score path.
    qT = sp_.tile([128, NTOK], BF16)          # q.T, rms-scaled
    kT0z = sp_.tile([128, NTOK], BF16)        # head0 k.T in rows 0-63, 0 pad
    kT1z = sp_.tile([128, NTOK], BF16)        # head1 k.T in rows 64-127
    nc.gpsimd.memset(kT0z[64:128, :], 0.0)
    nc.gpsimd.memset(kT1z[0:64, :], 0.0)
    vv0 = sp_.tile([128, 32, HD + 1], BF16)   # v (token-major) + ones col
    vv1 = sp_.tile([128, 32, HD + 1], BF16)
    nc.gpsimd.memset(vv0[:, :, HD:HD + 1], 1.0)
    nc.gpsimd.memset(vv1[:, :, HD:HD + 1], 1.0)
    wqh_sb = sp_.tile([128, NDC, E2], FP8)
    wql_sb = sp_.tile([128, NDC, E2], FP8)
    wkh_sb = sp_.tile([128, NDC, E2], FP8)
    wkl_sb = sp_.tile([128, NDC, E2], FP8)
    wvh_sb = sp_.tile([128, NDC, E2], FP8)
    wvl_sb = sp_.tile([128, NDC, E2], FP8)

    w1_tiles, w2_tiles = {}, {}

    def load_w1_oct(oc):
        # octant oc covers a-branch ilocs 4oc..4oc+3 (512 of 4096 ff dims);
        # bufs=3 so octants stay in flight without stalling the DMA queue
        # behind not-yet-consumed earlier octants
        w1h = p5.tile([128, NDC, 512], FP8, tag="w1h", bufs=3,
                      name=f"w1h{oc}")
        nc.sync.dma_start(
            out=w1h,
            in_=w1ahi_d.ap()[:, oc * 512:(oc + 1) * 512]
                .rearrange("(c p) m -> p c m", p=128))
        w1l = p5.tile([128, NDC, 512], FP8, tag="w1l", bufs=3,
                      name=f"w1l{oc}")
        nc.sync.dma_start(
            out=w1l,
            in_=w1alo_d.ap()[:, oc * 512:(oc + 1) * 512]
                .rearrange("(c p) m -> p c m", p=128))
        w1bt = p5.tile([128, NDC, 512], FP8, tag="w1b", bufs=3,
                       name=f"w1b{oc}")
        nc.sync.dma_start(
            out=w1bt,
            in_=w1b8_d.ap()[:, oc * 512:(oc + 1) * 512]
                .rearrange("(c p) m -> p c m", p=128))
        w1_tiles[oc] = (w1h, w1l, w1bt)

    def load_w2(qr):
        w2h = p5.tile([128, 8, D], FP8, tag="w2h", name=f"w2h{qr}")
        nc.sync.dma_start(
            out=w2h,
            in_=w2hi_d.ap()[qr * 1024:(qr + 1) * 1024, :]
                .rearrange("(i p) n -> p i n", p=128))
        w2l = p5.tile([128, 8, D], FP8, tag="w2l", name=f"w2l{qr}")
        nc.sync.dma_start(
            out=w2l,
            in_=w2lo_d.ap()[qr * 1024:(qr + 1) * 1024, :]
                .rearrange("(i p) n -> p i n", p=128))
        w2_tiles[qr] = (w2h, w2l)

    # ---- streaming pool (rolling per-chunk tiles) ----
    stream = tc.tile_pool(name="stream", bufs=1)
    st = stream.__enter__()
    attn = tc.tile_pool(name="attn", bufs=1)
    at = attn.__enter__()
    ps_cm = tc.tile_pool(name="psM", bufs=1, space="PSUM")
    ps = ps_cm.__enter__()
    # PSUM budget (8 banks): qk 1 + misc(ss/nbc/tp/bc) 1 + sp 4 + av 2

    xn_tiles = {}

    def dma_chunk(r):
        xnh = st.tile([128, NDC, 512], FP8, tag="xnh", bufs=3,
                      name=f"xnh{r}")
        xnl = st.tile([128, NDC, 512], FP8, tag="xnl", bufs=3,
                      name=f"xnl{r}")
        xn_tiles[r] = (xnh, xnl)
        if r == 0:
            # quarters, so the first Square can start after 0.12MB
            for cq in range(4):
                nc.sync.dma_start(
                    out=xnh[:, 2 * cq:2 * cq + 2, :],
                    in_=xth_d.ap()[256 * cq:256 * (cq + 1), 0:512]
                        .rearrange("(c p) t -> p c t", p=128))
        else:
            nc.sync.dma_start(
                out=xnh,
                in_=xth_d.ap()[:, r * 512:(r + 1) * 512]
                    .rearrange("(c p) t -> p c t", p=128))
        nc.sync.dma_start(
            out=xnl,
            in_=xtl_d.ap()[:, r * 512:(r + 1) * 512]
                .rearrange("(c p) t -> p c t", p=128))

    nbc_tiles = {}

    def stats_groups(r):
        """Emission thunks for chunk r's RMS stats, emitted one iteration
        AHEAD of chunk r's QKV so the serial Square->stats->Exp->nbc chain
        has a full iteration of slack before the qT/kT/vT muls need nbc.
        Squares are split in quarters so they slot between attention exps
        in the in-order Act queue without delaying a deferred AV."""
        xnh, xnl = xn_tiles[r]
        sq8 = st.tile([128, NDC, 512], FP8, tag="sq8", bufs=2,
                      name=f"sq8_{r}")
        ss_box = []
        gs = []

        # chunks whose stats land in exp-heavy iterations (6-8 attention
        # pairs in flight) square on DVE instead of Act: DVE runs fp8-out
        # in slow mode but has slack there, while Act is the binding engine.
        # xnh stores 16x, so Act pre-scales by 1/16 (sq8 = x^2) while the
        # DVE path computes (16x/16)*(16x) = 16 x^2 (fits fp8); the factor
        # is folded back in the lss scale below. Stats from the hi half
        # alone cost ~0.2% on the mean -- far below the bf16 rounding.
        sq_dve = r in (3, 4, 5)

        def g_stq(c2, sq8=sq8, xnh=xnh, r=r, sq_dve=sq_dve):
            def run(c2=c2):
                if sq_dve:
                    # (16x/32)*(16x) = 8x^2 <= ~242: 16x^2 would overflow
                    # fp8 (448) at |x|~5.3, which randn input does reach
                    nc.vector.scalar_tensor_tensor(
                        out=sq8[:, 2 * c2:2 * c2 + 2, :],
                        in0=xnh[:, 2 * c2:2 * c2 + 2, :],
                        scalar=1.0 / (2.0 * SX_SCALE),
                        in1=xnh[:, 2 * c2:2 * c2 + 2, :],
                        op0=ALU.mult, op1=ALU.mult)
                else:
                    nc.scalar.activation(out=sq8[:, 2 * c2:2 * c2 + 2, :],
                                         in_=xnh[:, 2 * c2:2 * c2 + 2, :],
                                         func=AF.Square,
                                         scale=1.0 / SX_SCALE)
                if c2 == 0:
                    ss = ps.tile([128, 512], F32, tag="misc", name=f"ss{r}")
                    ss_box.append(ss)
                nc.tensor.matmul(ss_box[0], lhsT=ones128f8,
                                 rhs=sq8[:, 2 * c2:2 * c2 + 2, :],
                                 start=(c2 == 0), stop=(c2 == 3),
                                 perf_mode=PM.DoubleRow)
            return run
        gs += [g_stq(c2) for c2 in range(4)]

        def g_rs(r=r):
            # nbc = exp(0.25*(v-2)^2 - 0.25) ~ rsqrt(v), v = ss/D + eps
            # (see epsm2_t comment); ss rows are all identical so this IS
            # already the partition-broadcast rms scale
            lss = st.tile([128, 512], F32, tag="lss", bufs=1,
                          name=f"lss{r}")
            ssc = 2.0 / (SX_SCALE * D) if r in (3, 4, 5) else 1.0 / D
            nc.scalar.activation(out=lss, in_=ss_box[0], func=AF.Square,
                                 bias=epsm2_t, scale=ssc)
            nbc_sb = st.tile([128, 512], BF16, tag="nbc", bufs=2,
                             name=f"nbcs{r}")
            nbc_tiles[r] = nbc_sb
            with nc.allow_low_precision(reason="rms scale bcast in bf16"):
                nc.scalar.activation(out=nbc_sb, in_=lss, func=AF.Exp,
                                     bias=mq_t, scale=0.25)
        gs.append(g_rs)
        return gs

    def chunk_groups(r):
        """Emission thunks for chunk r's QKV + v-transpose work (stats for
        chunk r were emitted last iteration)."""
        xnh, xnl = xn_tiles[r]
        gs = []

        def qkv_acc(name, wh, wl, xnh=xnh, xnl=xnl, r=r):
            # 3-term hi/lo fp8 DoubleRow: wh*xh + wh*xl + wl*xh (the
            # dropped wl*xl term is ~0.1%); 12 DR instrs at 2x rate vs
            # 8 bf16 instrs. Same [128,2,*] shape class as the FFN DRs.
            acc = ps.tile([128, 512], F32, tag="qk", name=name)
            terms = ((wh, xnh), (wh, xnl), (wl, xnh))
            for ti, (w_, x_) in enumerate(terms):
                for c2 in range(NDC // 2):
                    nc.tensor.matmul(
                        acc, lhsT=w_[:, 2 * c2:2 * c2 + 2, :],
                        rhs=x_[:, 2 * c2:2 * c2 + 2, :],
                        start=(ti == 0 and c2 == 0),
                        stop=(ti == 2 and c2 == NDC // 2 - 1),
                        perf_mode=PM.DoubleRow)
            return acc

        def g_q(r=r):
            acc = qkv_acc(f"qa{r}", wqh_sb, wql_sb)
            nc.vector.tensor_mul(out=qT[:, ts(r, 512)], in0=acc,
                                 in1=nbc_tiles[r])
        gs.append(g_q)

        def g_k(r=r):
            acc = qkv_acc(f"ka{r}", wkh_sb, wkl_sb)
            nbc_sb = nbc_tiles[r]
            nc.vector.tensor_mul(out=kT0z[0:64, ts(r, 512)],
                                 in0=acc[0:64, :], in1=nbc_sb[0:64, :])
            nc.vector.tensor_mul(out=kT1z[64:128, ts(r, 512)],
                                 in0=acc[64:128, :], in1=nbc_sb[64:128, :])
        gs.append(g_k)

        vtn_box = []

        def g_v(r=r):
            acc = qkv_acc(f"va{r}", wvh_sb, wvl_sb)
            vtn = st.tile([128, 512], BF16, tag="vtn", bufs=2,
                          name=f"vtn{r}")
            vtn_box.append(vtn)
            nc.vector.tensor_mul(out=vtn, in0=acc, in1=nbc_tiles[r])
        gs.append(g_v)

        def g_tp(r=r):
            vtn = vtn_box[0]
            for g in range(4):
                gt = r * 4 + g
                tp = ps.tile([128, 128], BF16, tag="misc", name=f"tp{gt}")
                nc.tensor.transpose(tp, vtn[:, ts(g, 128)], ident)
                nc.vector.tensor_copy(out=vv0[:, gt, 0:HD], in_=tp[:, 0:HD])
                nc.vector.tensor_copy(out=vv1[:, gt, 0:HD], in_=tp[:, HD:E2])
        gs.append(g_tp)
        return gs

    pending_norm = []

    def flush_norm():
        # deferred tail of softmax normalization: bc broadcast matmul +
        # chunk mul + a2a write, emitted only after the NEXT pair's score
        # matmuls so the in-order PE queue never stalls on the DVE chain
        while pending_norm:
            slot, h, o_sb, rc_t = pending_norm.pop(0)
            bc = ps.tile([HD, 512], F32, tag="misc", name=f"bc{slot}{h}")
            nc.tensor.matmul(bc, lhsT=ones65[64:65, :], rhs=rc_t[64:65, :])
            chunk = at.tile([HD, 512], BF16, tag="nrm", bufs=2)
            nc.vector.tensor_mul(out=chunk, in0=o_sb[0:HD, :], in1=bc)
            nc.sync.dma_start(
                out=a2a_in[slot, h * HD:(h + 1) * HD, :], in_=chunk)

    def make_pairs(c):
        """Attention pair thunks for chunk c (= slot c), 2 heads."""
        b, tq = c // 4, c % 4
        npair = 2 * (tq + 1)
        q0 = c * 512
        av0 = ps.tile([HD + 1, 512], F32, tag="av0", name=f"av0_{c}")
        av1 = ps.tile([HD + 1, 512], F32, tag="av1", name=f"av1_{c}")
        # diagonal pairs first: their narrow exp/mask chains pipeline
        # behind the full-width off-diagonal pairs that follow
        p_order = list(range(2 * tq, npair)) + list(range(0, 2 * tq))
        p_first, p_last = p_order[0], p_order[-1]
        av_pending = []

        def emit_av(keep=0):
            while len(av_pending) > keep:
                pp_, doff_, pe0_, pe1_ = av_pending.pop(0)
                g0 = (b * T) // 128 + 2 * pp_
                for j in range(2):
                    o = doff_[j]
                    nc.tensor.matmul(av0[:, o:512],
                                     lhsT=vv0[:, g0 + j, :],
                                     rhs=pe0_[:, j, o:512],
                                     start=(pp_ == p_first and j == 0),
                                     stop=(pp_ == p_last and j == 1))
                    nc.tensor.matmul(av1[:, o:512],
                                     lhsT=vv1[:, g0 + j, :],
                                     rhs=pe1_[:, j, o:512],
                                     start=(pp_ == p_first and j == 0),
                                     stop=(pp_ == p_last and j == 1))

        def pair_thunk(p):
            def run(p=p):
                k0 = b * T + p * 256
                s0 = ps.tile([128, 2, 512], F32, tag="sp", bufs=2,
                             name=f"s0_{c}_{p}")
                s1 = ps.tile([128, 2, 512], F32, tag="sp", bufs=2,
                             name=f"s1_{c}_{p}")
                doff = [max(0, (2 * p + j) - 4 * tq) * 128 if p >= 2 * tq
                        else 0 for j in range(2)]
                for j in range(2):
                    o = doff[j]
                    nc.tensor.matmul(
                        s0[:, j, o:512],
                        lhsT=kT0z[:, k0 + j * 128:k0 + j * 128 + 128],
                        rhs=qT[:, q0 + o:q0 + 512])
                    nc.tensor.matmul(
                        s1[:, j, o:512],
                        lhsT=kT1z[:, k0 + j * 128:k0 + j * 128 + 128],
                        rhs=qT[:, q0 + o:q0 + 512])
                emit_av(keep=1)    # drain to depth 1: deeper deferral
                pe0 = at.tile([128, 2, 512], BF16, tag="pe0", bufs=3,
                              name=f"pe0_{c}_{p}")
                pe1 = at.tile([128, 2, 512], BF16, tag="pe1", bufs=3,
                              name=f"pe1_{c}_{p}")
                if p >= 2 * tq:                  # diagonal pair
                    for pe, s in ((pe0, s0), (pe1, s1)):
                        for j in range(2):
                            o = doff[j]
                            nc.scalar.activation(out=pe[:, j, o:512],
                                                 in_=s[:, j, o:512],
                                                 func=AF.Exp,
                                                 scale=HD ** -0.5)
                            # block-local triangle mask on the first 128
                            # causal columns -- on Pool: the idle engine,
                            # and off the in-order DVE queue
                            nc.gpsimd.tensor_mul(out=pe[:, j, o:o + 128],
                                                 in0=pe[:, j, o:o + 128],
                                                 in1=mkt_sb)
                else:
                    nc.scalar.activation(out=pe0, in_=s0, func=AF.Exp,
                                         scale=HD ** -0.5)
                    nc.scalar.activation(out=pe1, in_=s1, func=AF.Exp,
                                         scale=HD ** -0.5)
                av_pending.append((p, doff, pe0, pe1))
                if p == p_first:
                    flush_norm()
            return run

        thunks = [pair_thunk(p) for p in p_order]

        def tail():
            emit_av()              # last pair's AV
            for h, av in ((0, av0), (1, av1)):
                o_sb = at.tile([HD + 1, 512], F32, tag=f"o{h}", bufs=2,
                               name=f"osb{c}{h}")
                nc.vector.tensor_copy(out=o_sb, in_=av)
                rc_t = at.tile([HD + 1, 512], BF16, tag=f"rc{h}", bufs=2,
                               name=f"rct{c}{h}")
                with nc.allow_low_precision(reason="softmax denom in bf16"):
                    nc.vector.reciprocal(out=rc_t[64:65, :],
                                         in_=o_sb[64:65, :])
                pending_norm.append((c, h, o_sb, rc_t))
        thunks.append(tail)
        return thunks

    # ===== merged streaming QKV + attention =====
    dma_chunk(0)
    for sb, d_ in ((wqh_sb, wqh_d), (wql_sb, wql_d), (wkh_sb, wkh_d),
                   (wkl_sb, wkl_d), (wvh_sb, wvh_d), (wvl_sb, wvl_d)):
        nc.sync.dma_start(out=sb,
                          in_=d_.ap().rearrange("(c p) m -> p c m", p=128))
    dma_chunk(1)

    for r in range(NRC + 1):
        groups = []
        if r < NRC:
            if r + 2 < NRC:
                dma_chunk(r + 2)
            # interleave later-phase loads behind the streaming chunks
            if r == 0:
                nc.sync.dma_start(out=mkt_sb, in_=mkt_d.ap())
                groups += stats_groups(0)
            elif r == 1:
                nc.sync.dma_start(
                    out=b1a_sb,
                    in_=b1_d.ap()[:4 * D].rearrange("(i p) -> p i", p=128))
                nc.sync.dma_start(
                    out=b1b_sb,
                    in_=b1_d.ap()[4 * D:].rearrange("(i p) -> p i", p=128))
            elif r == 2:
                nc.sync.dma_start(
                    out=xsp_sb,
                    in_=xsp_d.ap().rearrange("(tb p) n -> p tb n", p=128))
            elif r == 3:
                nc.sync.dma_start(
                    out=wp_sb,
                    in_=wp_d.ap().rearrange("(c p) n -> p c n", p=128))
            elif r == 4:
                nc.sync.dma_start(out=b2bc_sb, in_=b2bc_d.ap())
            elif r == 6:
                load_w1_oct(0)
            elif r == 7:
                load_w1_oct(1)
                load_w2(0)
            if r + 1 < NRC:
                groups += stats_groups(r + 1)
            groups += chunk_groups(r)
        pairs = make_pairs(r - 1) if r >= 1 else []
        # proportional interleave (Bresenham): attention exp latency hides
        # behind QKV/stats matmuls and vice versa
        ng, np_ = len(groups), len(pairs)
        gi = pi = 0
        while gi < ng or pi < np_:
            if pi * max(ng, 1) <= gi * max(np_, 1) and pi < np_:
                pairs[pi]()
                pi += 1
            elif gi < ng:
                groups[gi]()
                gi += 1
            else:
                pairs[pi]()
                pi += 1

    flush_norm()

    _finish(ps_cm, attn, stream, span)

    if stop_after < 3:
        _finish(p5_cm, dram, persist)
        return

    # ========== Phases 3-6: A2A, proj, RMS2, GLU FFN + down-proj ==========
    if variant in ("nocc",):
        nc.sync.dma_start(out=a2a_out[:], in_=a2a_in[:])
    else:
        nc.gpsimd.collective_compute(
            "AllToAll", ALU.bypass, replica_groups=[CORE_IDS],
            ins=[a2a_in[:].opt()], outs=[a2a_out[:].opt()])

    ffn = tc.tile_pool(name="ffn", bufs=1)
    fp = ffn.__enter__()
    xn2T8 = fp.tile([128, NDC, TS], FP8)
    gluhi = fp.tile([128, 32, TS], FP8)  # gluhi[p,i,t] = fp8(glu[t, i*128+p])
    glulo = fp.tile([128, 32, TS], FP8)  # fp8(glu - gluhi)

    p6_cm = tc.tile_pool(name="p6", bufs=2)
    p6 = p6_cm.__enter__()

    psc_cm = tc.tile_pool(name="psC", bufs=1, space="PSUM")
    psC = psc_cm.__enter__()

    # ---- projection (reads A2A output) ----
    with tc.tile_pool(name="proj", bufs=1) as pj:
        # one tile per source slot so proj matmuls gate on individual
        # slot arrivals instead of the whole 1MB load
        oT_s = []
        for s in range(NCORES):
            ot = pj.tile([128, TS], BF16, name=f"ot{s}")
            nc.sync.dma_start(out=ot, in_=a2a_out[s].rearrange("p f -> p f"))
            oT_s.append(ot)
        load_w2(1)
        load_w1_oct(2)

        def proj_tb(tb):
            for dt in range(D // 512):
                acc = psC.tile([128, 512], F32, tag="xo", bufs=4)
                for c in range(NDC):
                    nc.tensor.matmul(acc, lhsT=oT_s[c][:, ts(tb, 128)],
                                     rhs=wp_sb[:, c, ts(dt, 512)],
                                     start=(c == 0), stop=(c == NDC - 1))
                nc.vector.tensor_add(out=xsp_sb[:, tb, ts(dt, 512)],
                                     in0=acc, in1=xsp_sb[:, tb, ts(dt, 512)])

        def rms2_tb(tb):
            # second RMSNorm + on-chip PE transpose (fp8 out only)
            sq2 = p6.tile([128, D], BF16, tag="sq4")
            ssum = p6.tile([128, 1], F32, tag="ssum4")
            nc.scalar.activation(out=sq2, in_=xsp_sb[:, tb, :],
                                 func=AF.Square, accum_out=ssum)
            rs = p6.tile([128, 1], F32, tag="rs4")
            nc.scalar.activation(out=rs, in_=ssum, func=AF.Sqrt,
                                 bias=eps_t, scale=1.0 / D)
            nc.vector.reciprocal(out=rs, in_=rs)
            xn2_t = p6.tile([128, D], BF16, tag="xn2t")
            nc.vector.tensor_scalar_mul(out=xn2_t, in0=xsp_sb[:, tb, :],
                                        scalar1=rs)
            for cq in range(2):
                tpx = psC.tile([128, 4, 128], BF16, tag="tpx", bufs=2)
                for g in range(4):
                    c = cq * 4 + g
                    nc.tensor.transpose(tpx[:, g, :], xn2_t[:, ts(c, 128)],
                                        ident)
                # one strided copy per 4 transposes instead of 4 small ones
                nc.vector.tensor_copy(
                    out=xn2T8[:, 4 * cq:4 * cq + 4, ts(tb, 128)], in_=tpx)
            # fold b2 into the residual now that RMS2 has consumed xo
            nc.vector.tensor_add(out=xsp_sb[:, tb, :],
                                 in0=xsp_sb[:, tb, :], in1=b2bc_sb)

        # interleave: proj(tb+1)'s matmuls keep the PE busy while
        # rms2(tb)'s Act/DVE chain runs
        proj_tb(0)
        proj_tb(1)
        rms2_tb(0)
        proj_tb(2)
        rms2_tb(1)
        proj_tb(3)
        load_w1_oct(3)
        rms2_tb(2)
        rms2_tb(3)

    _finish(psc_cm)

    if stop_after < 5:
        _finish(p6_cm, ffn, p5_cm, dram, persist)
        return

    psd_cm = tc.tile_pool(name="psD", bufs=1, space="PSUM")
    psD = psd_cm.__enter__()

    # ---- GLU FFN with interleaved down-projection ----
    # (octants 0-3 + w2 quarter 0 were prefetched during earlier phases)
    for oc in range(8):
        w1h, w1l, w1bt = w1_tiles.pop(oc)
        for il in range(4):
            ia = oc * 4 + il
            # a-branch: hi+lo fp8 DoubleRow against fp8 xn2
            ha = psD.tile([128, TS], F32, tag="ha", bufs=2)
            first = True
            for wt in (w1h, w1l):
                for c2 in range(NDC // 2):
                    nc.tensor.matmul(
                        ha, lhsT=wt[:, 2 * c2:2 * c2 + 2, ts(il, 128)],
                        rhs=xn2T8[:, 2 * c2:2 * c2 + 2, :],
                        start=first,
                        stop=(wt is w1l and c2 == NDC // 2 - 1),
                        perf_mode=PM.DoubleRow)
                    first = False
            # aT = ha/W1A_SCALE + b1a  (Act Copy, frees DVE for glu chain)
            aT = p6.tile([128, TS], BF16, tag="aT")
            nc.scalar.activation(out=aT, in_=ha, func=AF.Identity,
                                 bias=b1a_sb[:, ia:ia + 1],
                                 scale=1.0 / W1A_SCALE)
            hb = psD.tile([128, TS], F32, tag="hb", bufs=2)
            for c2 in range(NDC // 2):
                nc.tensor.matmul(hb,
                                 lhsT=w1bt[:, 2 * c2:2 * c2 + 2, ts(il, 128)],
                                 rhs=xn2T8[:, 2 * c2:2 * c2 + 2, :],
                                 start=(c2 == 0), stop=(c2 == NDC // 2 - 1),
                                 perf_mode=PM.DoubleRow)
            sg = p6.tile([128, TS], BF16, tag="sg")
            nc.scalar.activation(out=sg, in_=hb, func=AF.Sigmoid,
                                 bias=b1b_sb[:, ia:ia + 1],
                                 scale=1.0 / W1B_SCALE)
            glu = p6.tile([128, TS], BF16, tag="glu")
            nc.vector.tensor_mul(out=glu, in0=aT, in1=sg)
            # hi/lo fp8 split of glu: hi cast on Pool (SBUF-only op),
            # lo = glu - hi on DVE
            nc.gpsimd.tensor_copy(out=gluhi[:, ia, :], in_=glu)
            nc.vector.tensor_tensor(out=glulo[:, ia, :], in0=glu,
                                    in1=gluhi[:, ia, :], op=ALU.subtract)
        # prefetch: next w1 octant / w2 quarter, emitted after this
        # octant's consumers so the DMA's WAR wait can't stall the queue
        if oc + 4 < 8:
            load_w1_oct(oc + 4)
        if oc == 3:
            load_w2(2)
        elif oc == 4:
            load_w2(3)
        if oc % 4 == 3:
            half = oc // 4
            w2ah, w2al = w2_tiles.pop(2 * half)
            w2bh, w2bl = w2_tiles.pop(2 * half + 1)
            for tb in range(TS // 128):
                for dt in range(D // 512):
                    yy = psD.tile([128, 512], F32, tag="yy", bufs=2)
                    first = True
                    # 3-term hi/lo product: ghi*whi + ghi*wlo + glo*whi
                    for gl, pick in ((gluhi, 0), (gluhi, 1), (glulo, 0)):
                        for jp in range(8):
                            i = half * 16 + 2 * jp
                            w2t = ((w2ah, w2bh) if pick == 0
                                   else (w2al, w2bl))[0 if jp < 4 else 1]
                            jloc = (2 * jp) % 8
                            nc.tensor.matmul(
                                yy, lhsT=gl[:, i:i + 2, ts(tb, 128)],
                                rhs=w2t[:, jloc:jloc + 2, ts(dt, 512)],
                                start=first,
                                stop=(gl is glulo and jp == 7),
                                perf_mode=PM.DoubleRow)
                            first = False
                    if half == 0:
                        nc.vector.scalar_tensor_tensor(
                            out=xsp_sb[:, tb, ts(dt, 512)], in0=yy,
                            scalar=1.0 / W2_SCALE,
                            in1=xsp_sb[:, tb, ts(dt, 512)],
                            op0=ALU.mult, op1=ALU.add)
                    else:
                        out_t = p6.tile([128, 512], F32, tag="out", bufs=2)
                        nc.vector.scalar_tensor_tensor(
                            out=out_t, in0=yy,
                            scalar=1.0 / W2_SCALE,
                            in1=xsp_sb[:, tb, ts(dt, 512)],
                            op0=ALU.mult, op1=ALU.add)
                        nc.sync.dma_start(
                            out=out_d.ap()[ts(tb, 128), ts(dt, 512)],
                            in_=out_t)

    _finish(psd_cm, p6_cm, ffn, p5_cm, dram, persist)


def _prep_inputs(x, wq, wk, wv, w_proj, b_proj, w1, b1, w2, b2, g1, g2):
    bf16 = ml_dtypes.bfloat16
    fp8 = ml_dtypes.float8_e4m3
    xf = np.asarray(x, np.float32).reshape(NTOK, D)
    g1 = np.asarray(g1, np.float32)
    g2 = np.asarray(g2, np.float32)
    def hilo(a, s):
        hi = (a * s).astype(fp8)
        lo = (a * s - hi.astype(np.float32)).astype(fp8)
        return hi, lo

    wqf = np.asarray(wq, np.float32) * g1[None, :, None]
    wkf = np.asarray(wk, np.float32) * g1[None, :, None]
    wvf = np.asarray(wv, np.float32) * g1[None, :, None]
    w1g = np.asarray(w1, np.float32) * g2[:, None]
    w1as = w1g[:, :4 * D] * W1A_SCALE
    w1ahi = w1as.astype(fp8)
    w1alo = np.ascontiguousarray(
        (w1as - w1ahi.astype(np.float32)).astype(fp8))
    w1ahi = np.ascontiguousarray(w1ahi)
    w1b8 = np.ascontiguousarray(
        (w1g[:, 4 * D:] * W1B_SCALE).astype(fp8))
    w2f = np.asarray(w2, np.float32) * W2_SCALE
    w2hi = w2f.astype(fp8)
    w2lo = np.ascontiguousarray((w2f - w2hi.astype(np.float32)).astype(fp8))
    w2hi = np.ascontiguousarray(w2hi)
    wpb = np.ascontiguousarray(np.asarray(w_proj, np.float32).astype(bf16))
    b1f = np.ascontiguousarray(np.asarray(b1, np.float32))
    b2bc = np.ascontiguousarray(np.broadcast_to(
        np.asarray(b2, np.float32).astype(bf16)[None, :], (128, D)))
    bp = np.asarray(b_proj, np.float32)
    xt8h, xt8l = hilo(xf.T, SX_SCALE)
    xt8h = np.ascontiguousarray(xt8h)
    xt8l = np.ascontiguousarray(xt8l)
    # block-local lower-triangle mask, shared by every diagonal block
    mkt = np.ascontiguousarray(
        (np.arange(128)[:, None] <= np.arange(128)[None, :]).astype(bf16))
    in_maps = []
    for c in range(NCORES):
        h0, h1 = HPC * c, HPC * c + 1
        wqc = np.concatenate([wqf[h0], wqf[h1]], 1)
        wkc = np.concatenate([wkf[h0], wkf[h1]], 1)
        wvc = np.concatenate([wvf[h0], wvf[h1]], 1)
        wq8h, wq8l = hilo(wqc, SW_SCALE)
        wk8h, wk8l = hilo(wkc, SW_SCALE)
        wv8h, wv8l = hilo(wvc, SW_SCALE)
        in_maps.append({
            "xt8h": xt8h,
            "xt8l": xt8l,
            "wq8h": np.ascontiguousarray(wq8h),
            "wq8l": np.ascontiguousarray(wq8l),
            "wk8h": np.ascontiguousarray(wk8h),
            "wk8l": np.ascontiguousarray(wk8l),
            "wv8h": np.ascontiguousarray(wv8h),
            "wv8l": np.ascontiguousarray(wv8l),
            "wp": wpb,
            "w1ahi": w1ahi,
            "w1alo": w1alo,
            "w1b8": w1b8,
            "b1": b1f,
            "w2hi": w2hi,
            "w2lo": w2lo,
            "b2bc": b2bc,
            "xsp": np.ascontiguousarray(xf[TS * c:TS * (c + 1)] + bp[None, :]),
            "mkt": mkt,
        })
    return in_maps


def kernel(**inputs):
    in_maps = _prep_inputs(**inputs)
    if "nc" not in _CACHE:
        _CACHE["nc"] = _build()
    res = run_bass_kernel_spmd(_CACHE["nc"], in_maps, CORE_IDS)
    out = np.concatenate([res.results[c]["out"] for c in range(NCORES)], 0)
    return out.reshape(B, T, D).astype(np.float32)


if __name__ == "__main__":
    import reference
    inputs = {k: np.asarray(v) for k, v in reference.setup_inputs().items()}
    got = kernel(**inputs)
    want = np.asarray(reference.reference(**inputs))
    err = np.abs(got - want)
    scale = np.abs(want).max()
    print("max abs err:", err.max(), "scale:", scale)
    print("rel err (max/scale):", err.max() / scale)


# revision 45
# speedup vs baseline: 2.0784x; 2.0784x over previous
"""Trainium2 Bass kernel for a dense transformer block (RMSNorm -> causal MHA
-> residual -> RMSNorm -> GLU FFN -> residual), SPMD across 8 NeuronCores.

Sharding: tensor-parallel attention (2 heads/core) -> AllToAll of per-head
attention outputs -> token-parallel proj + FFN (512 tokens/core).

v3 design (sim-driven; HW-verified constants from v2 retained).
HW-measured (loop-contrast k=2 vs 182): body 180.5us no-collective,
195.5us with the A2A estimate, vs the v2 baseline's 485us; rel-err
0.0145 vs the 2e-2 gate. TimelineSim (cold single pass) 276us; the
hardware loop pipelines across iterations so steady-state is lower.

 - the QKV streaming loop and causal attention are MERGED: chunk r's
   QKV/stats matmuls are emission-interleaved with chunk r-1's attention
   pairs, so the PE never idles waiting for the Act exp chain and stays
   at full p-state (2.4 GHz); attention alone is Act-exp-bound.
 - RMS1 stats run one chunk AHEAD of their QKV consumers (the serial
   Square->reduce->exp->scale chain gets a whole iteration of slack) and
   the DoubleRow ones-reduce uses M=128 (fp8 dual-row Ldweights with
   M=8 fails the s3_lw_dual_fp8_restrictions ISA check; M=128 also makes
   every ss row the full sum, so the partition broadcast is free).
 - the whole merged phase fits in 8 PSUM banks: per-head f32 score
   tiles (4) + one rotating QKV accumulator (1) + a shared
   ss/nbc/tp/bc scratch bank (1) + the two AV accumulators (2).
 - rs chain via exp(0.25*(v-2)^2-0.25) ~ rsqrt(v): Square+Exp share the
   attention exp act table -> no ACT_TABLE_LOADs inside the merged loop
   (Sqrt or Ln would force 2 reloads per chunk, ~24us total).
 - PSUM->SBUF copies (vv, xn2T8) on DVE, not Act (Act is the
   attention-critical engine; Pool/GpSimd has no PSUM access); diagonal
   mask muls on Pool (SBUF-only, keeps the in-order DVE queue free for
   the softmax-denominator tail so deferred AVs unblock sooner).
 - FFN: w1a as fp8 hi+lo DoubleRow against the existing fp8 xn2 (2
   passes, 2x rate), glu stored as fp8 hi+lo, w2 as 3-term hi/lo
   product. Offline numerics: rel-err 0.0144 vs 0.0135 baseline.
"""
import numpy as np
import ml_dtypes

import concourse.bass as bass
import concourse.tile as tile
from concourse import bacc, mybir
from concourse.bass_utils import run_bass_kernel_spmd
from concourse.masks import make_identity

F32 = mybir.dt.float32
BF16 = mybir.dt.bfloat16
FP8 = mybir.dt.float8e4
AF = mybir.ActivationFunctionType
ALU = mybir.AluOpType
PM = mybir.MatmulPerfMode

B, T, D, H, HD = 2, 2048, 1024, 16, 64
NCORES = 8
HPC = H // NCORES          # heads per core = 2
E2 = HPC * HD              # per-core attention channels = 128
NTOK = B * T               # 4096
TS = NTOK // NCORES        # tokens per core slice = 512
EPS = 1e-6
NDC = D // 128             # 8 D-chunks
NRC = NTOK // 512          # 8 token chunks
CORE_IDS = list(range(NCORES))
W1A_SCALE = 32.0           # fp8 pre-scale for w1 a-branch weights (hi+lo)
W1B_SCALE = 32.0           # fp8 pre-scale for w1 sigmoid-branch weights
W2_SCALE = 64.0            # fp8 pre-scale for w2 hi/lo split

_CACHE = {}


def _build(reps=1, variant="full", loop_k=0, stop_after=99):
    nc = bacc.Bacc("TRN2", target_bir_lowering=False, debug=False,
                   num_devices=NCORES)

    tensors = dict(
        xt=nc.dram_tensor("xt", [D, NTOK], BF16, kind="ExternalInput"),
        wq=nc.dram_tensor("wq", [D, E2], BF16, kind="ExternalInput"),
        wk=nc.dram_tensor("wk", [D, E2], BF16, kind="ExternalInput"),
        wv=nc.dram_tensor("wv", [D, E2], BF16, kind="ExternalInput"),
        wp=nc.dram_tensor("wp", [D, D], BF16, kind="ExternalInput"),
        w1ahi=nc.dram_tensor("w1ahi", [D, 4 * D], FP8, kind="ExternalInput"),
        w1alo=nc.dram_tensor("w1alo", [D, 4 * D], FP8, kind="ExternalInput"),
        w1b8=nc.dram_tensor("w1b8", [D, 4 * D], FP8, kind="ExternalInput"),
        b1=nc.dram_tensor("b1", [8 * D], F32, kind="ExternalInput"),
        w2hi=nc.dram_tensor("w2hi", [4 * D, D], FP8, kind="ExternalInput"),
        w2lo=nc.dram_tensor("w2lo", [4 * D, D], FP8, kind="ExternalInput"),
        b2bc=nc.dram_tensor("b2bc", [128, D], BF16, kind="ExternalInput"),
        xsp=nc.dram_tensor("xsp", [TS, D], F32, kind="ExternalInput"),
        mkt=nc.dram_tensor("mkt", [128, 128], BF16, kind="ExternalInput"),
        out=nc.dram_tensor("out", [TS, D], F32, kind="ExternalOutput"),
    )

    with tile.TileContext(nc) as tc:
        if loop_k:
            with tc.For_i(0, loop_k, 1):
                _body(nc, tc, tensors, variant=variant, stop_after=stop_after)
        else:
            for _ in range(reps):
                _body(nc, tc, tensors, variant=variant, stop_after=stop_after)
    nc.compile()
    return nc


def _finish(*pools):
    for p in pools:
        p.__exit__(None, None, None)


def _body(nc, tc, tn, variant="full", stop_after=99):
    ts = bass.ts
    xt_d, wq_d, wk_d, wv_d, wp_d = tn["xt"], tn["wq"], tn["wk"], tn["wv"], tn["wp"]
    w1ahi_d, w1alo_d, w1b8_d, b1_d = tn["w1ahi"], tn["w1alo"], tn["w1b8"], tn["b1"]
    w2hi_d, w2lo_d, b2bc_d = tn["w2hi"], tn["w2lo"], tn["b2bc"]
    xsp_d, mkt_d, out_d = tn["xsp"], tn["mkt"], tn["out"]

    persist = tc.tile_pool(name="persist", bufs=1)
    pp = persist.__enter__()
    dram = tc.tile_pool(name="dram", bufs=1, space="DRAM")
    dd = dram.__enter__()

    # ---- persistent small tensors ----
    eps_t = pp.tile([128, 1], F32)
    nc.vector.memset(eps_t, EPS)
    # warm the exp/square act table at t=0, overlapping the first DMAs
    # (otherwise the 1.3us ACT_TABLE_LOAD lands on the critical path)
    warm_t = pp.tile([128, 1], F32)
    nc.scalar.activation(out=warm_t, in_=eps_t, func=AF.Exp)
    # rs chain constants: recip8 = exp(0.25*(v-2)^2 - 0.25)/8 ~ rsqrt(v)/8
    # (3rd-order match at v=1; v=mean(x^2)+eps concentrates in [0.8,1.2] so
    # the error is ~t^3/6 < 2e-3, below the bf16 rounding of the result.
    # Uses only Square+Exp -> stays in the attention exp act table; Ln or
    # Sqrt here would force 2 ACT_TABLE_LOADs per chunk, ~24us total.)
    epsm2_t = pp.tile([128, 1], F32)
    nc.vector.memset(epsm2_t, EPS - 2.0)
    mq_t = pp.tile([128, 1], F32)
    nc.vector.memset(mq_t, -0.25)
    # stats DoubleRow lhsT (K=256, M=128): M=128 keeps the fp8 dual-row
    # Ldweights ISA-legal AND makes every row of ss the full column sum,
    # i.e. the partition broadcast comes for free
    ones128f8 = pp.tile([128, 2, 128], FP8)
    nc.vector.memset(ones128f8, 1.0)
    ones65 = pp.tile([65, HD], BF16)        # denom bcast lhsT at partition 64
    nc.vector.memset(ones65[64:65, :], 1.0)
    b1a_sb = pp.tile([128, 32], F32)
    b1b_sb = pp.tile([128, 32], F32)
    b2bc_sb = pp.tile([128, D], BF16)
    mkt_sb = pp.tile([128, 128], BF16)      # block-local triangle mask
    wp_sb = pp.tile([128, NDC, D], BF16)
    ident = pp.tile([128, 128], BF16)
    make_identity(nc, ident)
    # xsp_sb carries residual state through the whole kernel
    xsp_sb = pp.tile([128, 4, D], F32)

    a2a_in = dd.tile([NCORES, E2, TS], BF16)
    a2a_out = dd.tile([NCORES, E2, TS], BF16)

    # ---- FFN weight pool: opened FIRST (before the merged-phase pools)
    # so it can stay alive into the FFN while they close in LIFO order ----
    p5_cm = tc.tile_pool(name="p5", bufs=2)
    p5 = p5_cm.__enter__()

    # ---- tensors that span the merged phase ----
    span = tc.tile_pool(name="span", bufs=1)
    sp_ = span.__enter__()
    # NOTE: scores via fp8 DoubleRow (q/k packed as 64-partition k-tiles)
    # were tried and pass correctness (rel-err 0.0151) with -10us PE in
    # TimelineSim, but the looped HW build measured 1.53ms/body -- an
    # ~8x pathology the cost model does not capture (64-partition fp8
    # dual-row Ldweights or the SBUF->SBUF partition-remap DMAs).
    # Keeping the bf16 zero-padded score path.
    qT = sp_.tile([128, NTOK], BF16)          # q.T, rms-scaled
    kT0z = sp_.tile([128, NTOK], BF16)        # head0 k.T in rows 0-63, 0 pad
    kT1z = sp_.tile([128, NTOK], BF16)        # head1 k.T in rows 64-127
    nc.gpsimd.memset(kT0z[64:128, :], 0.0)
    nc.gpsimd.memset(kT1z[0:64, :], 0.0)
    vv0 = sp_.tile([128, 32, HD + 1], BF16)   # v (token-major) + ones col
    vv1 = sp_.tile([128, 32, HD + 1], BF16)
    nc.gpsimd.memset(vv0[:, :, HD:HD + 1], 1.0)
    nc.gpsimd.memset(vv1[:, :, HD:HD + 1], 1.0)
    wq_sb = sp_.tile([128, NDC, E2], BF16)
    wk_sb = sp_.tile([128, NDC, E2], BF16)
    wv_sb = sp_.tile([128, NDC, E2], BF16)

    w1_tiles, w2_tiles = {}, {}

    def load_w1_oct(oc):
        # octant oc covers a-branch ilocs 4oc..4oc+3 (512 of 4096 ff dims);
        # bufs=3 so octants stay in flight without stalling the DMA queue
        # behind not-yet-consumed earlier octants
        w1h = p5.tile([128, NDC, 512], FP8, tag="w1h", bufs=3,
                      name=f"w1h{oc}")
        nc.sync.dma_start(
            out=w1h,
            in_=w1ahi_d.ap()[:, oc * 512:(oc + 1) * 512]
                .rearrange("(c p) m -> p c m", p=128))
        w1l = p5.tile([128, NDC, 512], FP8, tag="w1l", bufs=3,
                      name=f"w1l{oc}")
        nc.sync.dma_start(
            out=w1l,
            in_=w1alo_d.ap()[:, oc * 512:(oc + 1) * 512]
                .rearrange("(c p) m -> p c m", p=128))
        w1bt = p5.tile([128, NDC, 512], FP8, tag="w1b", bufs=3,
                       name=f"w1b{oc}")
        nc.sync.dma_start(
            out=w1bt,
            in_=w1b8_d.ap()[:, oc * 512:(oc + 1) * 512]
                .rearrange("(c p) m -> p c m", p=128))
        w1_tiles[oc] = (w1h, w1l, w1bt)

    def load_w2(qr):
        w2h = p5.tile([128, 8, D], FP8, tag="w2h", name=f"w2h{qr}")
        nc.sync.dma_start(
            out=w2h,
            in_=w2hi_d.ap()[qr * 1024:(qr + 1) * 1024, :]
                .rearrange("(i p) n -> p i n", p=128))
        w2l = p5.tile([128, 8, D], FP8, tag="w2l", name=f"w2l{qr}")
        nc.sync.dma_start(
            out=w2l,
            in_=w2lo_d.ap()[qr * 1024:(qr + 1) * 1024, :]
                .rearrange("(i p) n -> p i n", p=128))
        w2_tiles[qr] = (w2h, w2l)

    # ---- streaming pool (rolling per-chunk tiles) ----
    stream = tc.tile_pool(name="stream", bufs=1)
    st = stream.__enter__()
    attn = tc.tile_pool(name="attn", bufs=1)
    at = attn.__enter__()
    ps_cm = tc.tile_pool(name="psM", bufs=1, space="PSUM")
    ps = ps_cm.__enter__()
    # PSUM budget (8 banks): qk 1 + misc(ss/nbc/tp/bc) 1 + sp 4 + av 2

    xn_tiles = {}

    def dma_chunk(r):
        xn = st.tile([128, NDC, 512], BF16, tag="xn", bufs=3, name=f"xn{r}")
        xn_tiles[r] = xn
        if r == 0:
            # quarters, so the first Square can start after 0.25MB
            for cq in range(4):
                nc.sync.dma_start(
                    out=xn[:, 2 * cq:2 * cq + 2, :],
                    in_=xt_d.ap()[256 * cq:256 * (cq + 1), 0:512]
                        .rearrange("(c p) t -> p c t", p=128))
        else:
            nc.sync.dma_start(
                out=xn,
                in_=xt_d.ap()[:, r * 512:(r + 1) * 512]
                    .rearrange("(c p) t -> p c t", p=128))

    nbc_tiles = {}

    def stats_groups(r):
        """Emission thunks for chunk r's RMS stats, emitted one iteration
        AHEAD of chunk r's QKV so the serial Square->stats->Exp->nbc chain
        has a full iteration of slack before the qT/kT/vT muls need nbc.
        Squares are split in quarters so they slot between attention exps
        in the in-order Act queue without delaying a deferred AV."""
        xn = xn_tiles[r]
        sq8 = st.tile([128, NDC, 512], FP8, tag="sq8", bufs=2,
                      name=f"sq8_{r}")
        ss_box = []
        gs = []

        # chunks whose stats land in exp-heavy iterations (6-8 attention
        # pairs in flight) square on DVE instead of Act: DVE runs fp8-out
        # in slow mode but has slack there, while Act is the binding engine
        sq_dve = r in (3, 4, 5)

        def g_stq(c2, sq8=sq8, xn=xn, r=r, sq_dve=sq_dve):
            def run(c2=c2):
                if sq_dve:
                    nc.vector.tensor_mul(out=sq8[:, 2 * c2:2 * c2 + 2, :],
                                         in0=xn[:, 2 * c2:2 * c2 + 2, :],
                                         in1=xn[:, 2 * c2:2 * c2 + 2, :])
                else:
                    nc.scalar.activation(out=sq8[:, 2 * c2:2 * c2 + 2, :],
                                         in_=xn[:, 2 * c2:2 * c2 + 2, :],
                                         func=AF.Square)
                if c2 == 0:
                    ss = ps.tile([128, 512], F32, tag="misc", name=f"ss{r}")
                    ss_box.append(ss)
                nc.tensor.matmul(ss_box[0], lhsT=ones128f8,
                                 rhs=sq8[:, 2 * c2:2 * c2 + 2, :],
                                 start=(c2 == 0), stop=(c2 == 3),
                                 perf_mode=PM.DoubleRow)
            return run
        gs += [g_stq(c2) for c2 in range(4)]

        def g_rs(r=r):
            # nbc = exp(0.25*(v-2)^2 - 0.25) ~ rsqrt(v), v = ss/D + eps
            # (see epsm2_t comment); ss rows are all identical so this IS
            # already the partition-broadcast rms scale
            lss = st.tile([128, 512], F32, tag="lss", bufs=1,
                          name=f"lss{r}")
            nc.scalar.activation(out=lss, in_=ss_box[0], func=AF.Square,
                                 bias=epsm2_t, scale=1.0 / D)
            nbc_sb = st.tile([128, 512], BF16, tag="nbc", bufs=2,
                             name=f"nbcs{r}")
            nbc_tiles[r] = nbc_sb
            with nc.allow_low_precision(reason="rms scale bcast in bf16"):
                nc.scalar.activation(out=nbc_sb, in_=lss, func=AF.Exp,
                                     bias=mq_t, scale=0.25)
        gs.append(g_rs)
        return gs

    def chunk_groups(r):
        """Emission thunks for chunk r's QKV + v-transpose work (stats for
        chunk r were emitted last iteration)."""
        xn = xn_tiles[r]
        gs = []

        def g_q(xn=xn, r=r):
            acc = ps.tile([128, 512], F32, tag="qk", name=f"qa{r}")
            for c in range(NDC):
                nc.tensor.matmul(acc, lhsT=wq_sb[:, c, :], rhs=xn[:, c, :],
                                 start=(c == 0), stop=(c == NDC - 1))
            nc.vector.tensor_mul(out=qT[:, ts(r, 512)], in0=acc,
                                 in1=nbc_tiles[r])
        gs.append(g_q)

        def g_k(xn=xn, r=r):
            acc = ps.tile([128, 512], F32, tag="qk", name=f"ka{r}")
            for c in range(NDC):
                nc.tensor.matmul(acc, lhsT=wk_sb[:, c, :], rhs=xn[:, c, :],
                                 start=(c == 0), stop=(c == NDC - 1))
            nbc_sb = nbc_tiles[r]
            nc.vector.tensor_mul(out=kT0z[0:64, ts(r, 512)],
                                 in0=acc[0:64, :], in1=nbc_sb[0:64, :])
            nc.vector.tensor_mul(out=kT1z[64:128, ts(r, 512)],
                                 in0=acc[64:128, :], in1=nbc_sb[64:128, :])
        gs.append(g_k)

        vtn_box = []

        def g_v(xn=xn, r=r):
            acc = ps.tile([128, 512], F32, tag="qk", name=f"va{r}")
            for c in range(NDC):
                nc.tensor.matmul(acc, lhsT=wv_sb[:, c, :], rhs=xn[:, c, :],
                                 start=(c == 0), stop=(c == NDC - 1))
            vtn = st.tile([128, 512], BF16, tag="vtn", bufs=2,
                          name=f"vtn{r}")
            vtn_box.append(vtn)
            nc.vector.tensor_mul(out=vtn, in0=acc, in1=nbc_tiles[r])
        gs.append(g_v)

        def g_tp(r=r):
            vtn = vtn_box[0]
            for g in range(4):
                gt = r * 4 + g
                tp = ps.tile([128, 128], BF16, tag="misc", name=f"tp{gt}")
                nc.tensor.transpose(tp, vtn[:, ts(g, 128)], ident)
                nc.vector.tensor_copy(out=vv0[:, gt, 0:HD], in_=tp[:, 0:HD])
                nc.vector.tensor_copy(out=vv1[:, gt, 0:HD], in_=tp[:, HD:E2])
        gs.append(g_tp)
        return gs

    pending_norm = []

    def flush_norm():
        # deferred tail of softmax normalization: bc broadcast matmul +
        # chunk mul + a2a write, emitted only after the NEXT pair's score
        # matmuls so the in-order PE queue never stalls on the DVE chain
        while pending_norm:
            slot, h, o_sb, rc_t = pending_norm.pop(0)
            bc = ps.tile([HD, 512], F32, tag="misc", name=f"bc{slot}{h}")
            nc.tensor.matmul(bc, lhsT=ones65[64:65, :], rhs=rc_t[64:65, :])
            chunk = at.tile([HD, 512], BF16, tag="nrm", bufs=2)
            nc.vector.tensor_mul(out=chunk, in0=o_sb[0:HD, :], in1=bc)
            nc.sync.dma_start(
                out=a2a_in[slot, h * HD:(h + 1) * HD, :], in_=chunk)

    def make_pairs(c):
        """Attention pair thunks for chunk c (= slot c), 2 heads."""
        b, tq = c // 4, c % 4
        npair = 2 * (tq + 1)
        q0 = c * 512
        av0 = ps.tile([HD + 1, 512], F32, tag="av0", name=f"av0_{c}")
        av1 = ps.tile([HD + 1, 512], F32, tag="av1", name=f"av1_{c}")
        # diagonal pairs first: their narrow exp/mask chains pipeline
        # behind the full-width off-diagonal pairs that follow
        p_order = list(range(2 * tq, npair)) + list(range(0, 2 * tq))
        p_first, p_last = p_order[0], p_order[-1]
        av_pending = []

        def emit_av(keep=0):
            while len(av_pending) > keep:
                pp_, doff_, pe0_, pe1_ = av_pending.pop(0)
                g0 = (b * T) // 128 + 2 * pp_
                for j in range(2):
                    o = doff_[j]
                    nc.tensor.matmul(av0[:, o:512],
                                     lhsT=vv0[:, g0 + j, :],
                                     rhs=pe0_[:, j, o:512],
                                     start=(pp_ == p_first and j == 0),
                                     stop=(pp_ == p_last and j == 1))
                    nc.tensor.matmul(av1[:, o:512],
                                     lhsT=vv1[:, g0 + j, :],
                                     rhs=pe1_[:, j, o:512],
                                     start=(pp_ == p_first and j == 0),
                                     stop=(pp_ == p_last and j == 1))

        def pair_thunk(p):
            def run(p=p):
                k0 = b * T + p * 256
                s0 = ps.tile([128, 2, 512], F32, tag="sp", bufs=2,
                             name=f"s0_{c}_{p}")
                s1 = ps.tile([128, 2, 512], F32, tag="sp", bufs=2,
                             name=f"s1_{c}_{p}")
                doff = [max(0, (2 * p + j) - 4 * tq) * 128 if p >= 2 * tq
                        else 0 for j in range(2)]
                for j in range(2):
                    o = doff[j]
                    nc.tensor.matmul(
                        s0[:, j, o:512],
                        lhsT=kT0z[:, k0 + j * 128:k0 + j * 128 + 128],
                        rhs=qT[:, q0 + o:q0 + 512])
                    nc.tensor.matmul(
                        s1[:, j, o:512],
                        lhsT=kT1z[:, k0 + j * 128:k0 + j * 128 + 128],
                        rhs=qT[:, q0 + o:q0 + 512])
                emit_av(keep=1)    # drain to depth 1: deeper deferral
                pe0 = at.tile([128, 2, 512], BF16, tag="pe0", bufs=3,
                              name=f"pe0_{c}_{p}")
                pe1 = at.tile([128, 2, 512], BF16, tag="pe1", bufs=3,
                              name=f"pe1_{c}_{p}")
                if p >= 2 * tq:                  # diagonal pair
                    for pe, s in ((pe0, s0), (pe1, s1)):
                        for j in range(2):
                            o = doff[j]
                            nc.scalar.activation(out=pe[:, j, o:512],
                                                 in_=s[:, j, o:512],
                                                 func=AF.Exp,
                                                 scale=HD ** -0.5)
                            # block-local triangle mask on the first 128
                            # causal columns -- on Pool: the idle engine,
                            # and off the in-order DVE queue
                            nc.gpsimd.tensor_mul(out=pe[:, j, o:o + 128],
                                                 in0=pe[:, j, o:o + 128],
                                                 in1=mkt_sb)
                else:
                    nc.scalar.activation(out=pe0, in_=s0, func=AF.Exp,
                                         scale=HD ** -0.5)
                    nc.scalar.activation(out=pe1, in_=s1, func=AF.Exp,
                                         scale=HD ** -0.5)
                av_pending.append((p, doff, pe0, pe1))
                if p == p_first:
                    flush_norm()
            return run

        thunks = [pair_thunk(p) for p in p_order]

        def tail():
            emit_av()              # last pair's AV
            for h, av in ((0, av0), (1, av1)):
                o_sb = at.tile([HD + 1, 512], F32, tag=f"o{h}", bufs=2,
                               name=f"osb{c}{h}")
                nc.vector.tensor_copy(out=o_sb, in_=av)
                rc_t = at.tile([HD + 1, 512], BF16, tag=f"rc{h}", bufs=2,
                               name=f"rct{c}{h}")
                with nc.allow_low_precision(reason="softmax denom in bf16"):
                    nc.vector.reciprocal(out=rc_t[64:65, :],
                                         in_=o_sb[64:65, :])
                pending_norm.append((c, h, o_sb, rc_t))
        thunks.append(tail)
        return thunks

    # ===== merged streaming QKV + attention =====
    dma_chunk(0)
    nc.sync.dma_start(out=wq_sb, in_=wq_d.ap().rearrange("(c p) m -> p c m",
                                                         p=128))
    nc.sync.dma_start(out=wk_sb, in_=wk_d.ap().rearrange("(c p) m -> p c m",
                                                         p=128))
    nc.sync.dma_start(out=wv_sb, in_=wv_d.ap().rearrange("(c p) m -> p c m",
                                                         p=128))
    dma_chunk(1)

    for r in range(NRC + 1):
        groups = []
        if r < NRC:
            if r + 2 < NRC:
                dma_chunk(r + 2)
            # interleave later-phase loads behind the streaming chunks
            if r == 0:
                nc.sync.dma_start(out=mkt_sb, in_=mkt_d.ap())
                groups += stats_groups(0)
            elif r == 1:
                nc.sync.dma_start(
                    out=b1a_sb,
                    in_=b1_d.ap()[:4 * D].rearrange("(i p) -> p i", p=128))
                nc.sync.dma_start(
                    out=b1b_sb,
                    in_=b1_d.ap()[4 * D:].rearrange("(i p) -> p i", p=128))
            elif r == 2:
                nc.sync.dma_start(
                    out=xsp_sb,
                    in_=xsp_d.ap().rearrange("(tb p) n -> p tb n", p=128))
            elif r == 3:
                nc.sync.dma_start(
                    out=wp_sb,
                    in_=wp_d.ap().rearrange("(c p) n -> p c n", p=128))
            elif r == 4:
                nc.sync.dma_start(out=b2bc_sb, in_=b2bc_d.ap())
            elif r == 6:
                load_w1_oct(0)
            elif r == 7:
                load_w1_oct(1)
                load_w2(0)
            if r + 1 < NRC:
                groups += stats_groups(r + 1)
            groups += chunk_groups(r)
        pairs = make_pairs(r - 1) if r >= 1 else []
        # proportional interleave (Bresenham): attention exp latency hides
        # behind QKV/stats matmuls and vice versa
        ng, np_ = len(groups), len(pairs)
        gi = pi = 0
        while gi < ng or pi < np_:
            if pi * max(ng, 1) <= gi * max(np_, 1) and pi < np_:
                pairs[pi]()
                pi += 1
            elif gi < ng:
                groups[gi]()
                gi += 1
            else:
                pairs[pi]()
                pi += 1

    flush_norm()

    _finish(ps_cm, attn, stream, span)

    if stop_after < 3:
        _finish(p5_cm, dram, persist)
        return

    # ========== Phases 3-6: A2A, proj, RMS2, GLU FFN + down-proj ==========
    if variant in ("nocc",):
        nc.sync.dma_start(out=a2a_out[:], in_=a2a_in[:])
    else:
        nc.gpsimd.collective_compute(
            "AllToAll", ALU.bypass, replica_groups=[CORE_IDS],
            ins=[a2a_in[:].opt()], outs=[a2a_out[:].opt()])

    ffn = tc.tile_pool(name="ffn", bufs=1)
    fp = ffn.__enter__()
    xn2T8 = fp.tile([128, NDC, TS], FP8)
    gluhi = fp.tile([128, 32, TS], FP8)  # gluhi[p,i,t] = fp8(glu[t, i*128+p])
    glulo = fp.tile([128, 32, TS], FP8)  # fp8(glu - gluhi)

    p6_cm = tc.tile_pool(name="p6", bufs=2)
    p6 = p6_cm.__enter__()

    psc_cm = tc.tile_pool(name="psC", bufs=1, space="PSUM")
    psC = psc_cm.__enter__()

    # ---- projection (reads A2A output) ----
    with tc.tile_pool(name="proj", bufs=1) as pj:
        # one tile per source slot so proj matmuls gate on individual
        # slot arrivals instead of the whole 1MB load
        oT_s = []
        for s in range(NCORES):
            ot = pj.tile([128, TS], BF16, name=f"ot{s}")
            nc.sync.dma_start(out=ot, in_=a2a_out[s].rearrange("p f -> p f"))
            oT_s.append(ot)
        load_w2(1)
        load_w1_oct(2)

        def proj_tb(tb):
            for dt in range(D // 512):
                acc = psC.tile([128, 512], F32, tag="xo", bufs=4)
                for c in range(NDC):
                    nc.tensor.matmul(acc, lhsT=oT_s[c][:, ts(tb, 128)],
                                     rhs=wp_sb[:, c, ts(dt, 512)],
                                     start=(c == 0), stop=(c == NDC - 1))
                nc.vector.tensor_add(out=xsp_sb[:, tb, ts(dt, 512)],
                                     in0=acc, in1=xsp_sb[:, tb, ts(dt, 512)])

        def rms2_tb(tb):
            # second RMSNorm + on-chip PE transpose (fp8 out only)
            sq2 = p6.tile([128, D], BF16, tag="sq4")
            ssum = p6.tile([128, 1], F32, tag="ssum4")
            nc.scalar.activation(out=sq2, in_=xsp_sb[:, tb, :],
                                 func=AF.Square, accum_out=ssum)
            rs = p6.tile([128, 1], F32, tag="rs4")
            nc.scalar.activation(out=rs, in_=ssum, func=AF.Sqrt,
                                 bias=eps_t, scale=1.0 / D)
            nc.vector.reciprocal(out=rs, in_=rs)
            xn2_t = p6.tile([128, D], BF16, tag="xn2t")
            nc.vector.tensor_scalar_mul(out=xn2_t, in0=xsp_sb[:, tb, :],
                                        scalar1=rs)
            for cq in range(2):
                tpx = psC.tile([128, 4, 128], BF16, tag="tpx", bufs=2)
                for g in range(4):
                    c = cq * 4 + g
                    nc.tensor.transpose(tpx[:, g, :], xn2_t[:, ts(c, 128)],
                                        ident)
                # one strided copy per 4 transposes instead of 4 small ones
                nc.vector.tensor_copy(
                    out=xn2T8[:, 4 * cq:4 * cq + 4, ts(tb, 128)], in_=tpx)
            # fold b2 into the residual now that RMS2 has consumed xo
            nc.vector.tensor_add(out=xsp_sb[:, tb, :],
                                 in0=xsp_sb[:, tb, :], in1=b2bc_sb)

        # interleave: proj(tb+1)'s matmuls keep the PE busy while
        # rms2(tb)'s Act/DVE chain runs
        proj_tb(0)
        proj_tb(1)
        rms2_tb(0)
        proj_tb(2)
        rms2_tb(1)
        proj_tb(3)
        load_w1_oct(3)
        rms2_tb(2)
        rms2_tb(3)

    _finish(psc_cm)

    if stop_after < 5:
        _finish(p6_cm, ffn, p5_cm, dram, persist)
        return

    psd_cm = tc.tile_pool(name="psD", bufs=1, space="PSUM")
    psD = psd_cm.__enter__()

    # ---- GLU FFN with interleaved down-projection ----
    # (octants 0-3 + w2 quarter 0 were prefetched during earlier phases)
    for oc in range(8):
        w1h, w1l, w1bt = w1_tiles.pop(oc)
        for il in range(4):
            ia = oc * 4 + il
            # a-branch: hi+lo fp8 DoubleRow against fp8 xn2
            ha = psD.tile([128, TS], F32, tag="ha", bufs=2)
            first = True
            for wt in (w1h, w1l):
                for c2 in range(NDC // 2):
                    nc.tensor.matmul(
                        ha, lhsT=wt[:, 2 * c2:2 * c2 + 2, ts(il, 128)],
                        rhs=xn2T8[:, 2 * c2:2 * c2 + 2, :],
                        start=first,
                        stop=(wt is w1l and c2 == NDC // 2 - 1),
                        perf_mode=PM.DoubleRow)
                    first = False
            # aT = ha/W1A_SCALE + b1a  (Act Copy, frees DVE for glu chain)
            aT = p6.tile([128, TS], BF16, tag="aT")
            nc.scalar.activation(out=aT, in_=ha, func=AF.Identity,
                                 bias=b1a_sb[:, ia:ia + 1],
                                 scale=1.0 / W1A_SCALE)
            hb = psD.tile([128, TS], F32, tag="hb", bufs=2)
            for c2 in range(NDC // 2):
                nc.tensor.matmul(hb,
                                 lhsT=w1bt[:, 2 * c2:2 * c2 + 2, ts(il, 128)],
                                 rhs=xn2T8[:, 2 * c2:2 * c2 + 2, :],
                                 start=(c2 == 0), stop=(c2 == NDC // 2 - 1),
                                 perf_mode=PM.DoubleRow)
            sg = p6.tile([128, TS], BF16, tag="sg")
            nc.scalar.activation(out=sg, in_=hb, func=AF.Sigmoid,
                                 bias=b1b_sb[:, ia:ia + 1],
                                 scale=1.0 / W1B_SCALE)
            glu = p6.tile([128, TS], BF16, tag="glu")
            nc.vector.tensor_mul(out=glu, in0=aT, in1=sg)
            # hi/lo fp8 split of glu: hi cast on Pool (SBUF-only op),
            # lo = glu - hi on DVE
            nc.gpsimd.tensor_copy(out=gluhi[:, ia, :], in_=glu)
            nc.vector.tensor_tensor(out=glulo[:, ia, :], in0=glu,
                                    in1=gluhi[:, ia, :], op=ALU.subtract)
        # prefetch: next w1 octant / w2 quarter, emitted after this
        # octant's consumers so the DMA's WAR wait can't stall the queue
        if oc + 4 < 8:
            load_w1_oct(oc + 4)
        if oc == 3:
            load_w2(2)
        elif oc == 4:
            load_w2(3)
        if oc % 4 == 3:
            half = oc // 4
            w2ah, w2al = w2_tiles.pop(2 * half)
            w2bh, w2bl = w2_tiles.pop(2 * half + 1)
            for tb in range(TS // 128):
                for dt in range(D // 512):
                    yy = psD.tile([128, 512], F32, tag="yy", bufs=2)
                    first = True
                    # 3-term hi/lo product: ghi*whi + ghi*wlo + glo*whi
                    for gl, pick in ((gluhi, 0), (gluhi, 1), (glulo, 0)):
                        for jp in range(8):
                            i = half * 16 + 2 * jp
                            w2t = ((w2ah, w2bh) if pick == 0
                                   else (w2al, w2bl))[0 if jp < 4 else 1]
                            jloc = (2 * jp) % 8
                            nc.tensor.matmul(
                                yy, lhsT=gl[:, i:i + 2, ts(tb, 128)],
                                rhs=w2t[:, jloc:jloc + 2, ts(dt, 512)],
                                start=first,
                                stop=(gl is glulo and jp == 7),
                                perf_mode=PM.DoubleRow)
                            first = False
                    if half == 0:
                        nc.vector.scalar_tensor_tensor(
                            out=xsp_sb[:, tb, ts(dt, 512)], in0=yy,
                            scalar=1.0 / W2_SCALE,
                            in1=xsp_sb[:, tb, ts(dt, 512)],
                            op0=ALU.mult, op1=ALU.add)
                    else:
                        out_t = p6.tile([128, 512], F32, tag="out", bufs=2)
                        nc.vector.scalar_tensor_tensor(
                            out=out_t, in0=yy,
                            scalar=1.0 / W2_SCALE,
                            in1=xsp_sb[:, tb, ts(dt, 512)],
                            op0=ALU.mult, op1=ALU.add)
                        nc.sync.dma_start(
                            out=out_d.ap()[ts(tb, 128), ts(dt, 512)],
                            in_=out_t)

    _finish(psd_cm, p6_cm, ffn, p5_cm, dram, persist)


def _prep_inputs(x, wq, wk, wv, w_proj, b_proj, w1, b1, w2, b2, g1, g2):
    bf16 = ml_dtypes.bfloat16
    fp8 = ml_dtypes.float8_e4m3
    xf = np.asarray(x, np.float32).reshape(NTOK, D)
    g1 = np.asarray(g1, np.float32)
    g2 = np.asarray(g2, np.float32)
    wqf = (np.asarray(wq, np.float32) * g1[None, :, None]).astype(bf16)
    wkf = (np.asarray(wk, np.float32) * g1[None, :, None]).astype(bf16)
    wvf = (np.asarray(wv, np.float32) * g1[None, :, None]).astype(bf16)
    w1g = np.asarray(w1, np.float32) * g2[:, None]
    w1as = w1g[:, :4 * D] * W1A_SCALE
    w1ahi = w1as.astype(fp8)
    w1alo = np.ascontiguousarray(
        (w1as - w1ahi.astype(np.float32)).astype(fp8))
    w1ahi = np.ascontiguousarray(w1ahi)
    w1b8 = np.ascontiguousarray(
        (w1g[:, 4 * D:] * W1B_SCALE).astype(fp8))
    w2f = np.asarray(w2, np.float32) * W2_SCALE
    w2hi = w2f.astype(fp8)
    w2lo = np.ascontiguousarray((w2f - w2hi.astype(np.float32)).astype(fp8))
    w2hi = np.ascontiguousarray(w2hi)
    wpb = np.ascontiguousarray(np.asarray(w_proj, np.float32).astype(bf16))
    b1f = np.ascontiguousarray(np.asarray(b1, np.float32))
    b2bc = np.ascontiguousarray(np.broadcast_to(
        np.asarray(b2, np.float32).astype(bf16)[None, :], (128, D)))
    bp = np.asarray(b_proj, np.float32)
    xtr = np.ascontiguousarray(xf.T.astype(bf16))
    # block-local lower-triangle mask, shared by every diagonal block
    mkt = np.ascontiguousarray(
        (np.arange(128)[:, None] <= np.arange(128)[None, :]).astype(bf16))
    in_maps = []
    for c in range(NCORES):
        h0, h1 = HPC * c, HPC * c + 1
        in_maps.append({
            "xt": xtr,
            "wq": np.ascontiguousarray(np.concatenate([wqf[h0], wqf[h1]], 1)),
            "wk": np.ascontiguousarray(np.concatenate([wkf[h0], wkf[h1]], 1)),
            "wv": np.ascontiguousarray(np.concatenate([wvf[h0], wvf[h1]], 1)),
            "wp": wpb,
            "w1ahi": w1ahi,
            "w1alo": w1alo,
            "w1b8": w1b8,
            "b1": b1f,
            "w2hi": w2hi,
            "w2lo": w2lo,
            "b2bc": b2bc,
            "xsp": np.ascontiguousarray(xf[TS * c:TS * (c + 1)] + bp[None, :]),
            "mkt": mkt,
        })
    return in_maps


def kernel(**inputs):
    in_maps = _prep_inputs(**inputs)
    if "nc" not in _CACHE:
        _CACHE["nc"] = _build()
    res = run_bass_kernel_spmd(_CACHE["nc"], in_maps, CORE_IDS)
    out = np.concatenate([res.results[c]["out"] for c in range(NCORES)], 0)
    return out.reshape(B, T, D).astype(np.float32)


if __name__ == "__main__":
    import reference
    inputs = {k: np.asarray(v) for k, v in reference.setup_inputs().items()}
    got = kernel(**inputs)
    want = np.asarray(reference.reference(**inputs))
    err = np.abs(got - want)
    scale = np.abs(want).max()
    print("max abs err:", err.max(), "scale:", scale)
    print("rel err (max/scale):", err.max() / scale)
